# revision 18
# baseline (speedup 1.0000x reference)
"""RGCN-with-history (DGL RelGraphConv + history splice) on 8 TRN2 NeuronCores.

Key structural fact: the history splice dominates — out[n] is an exact copy of
history_buffer[history_map[n]] wherever history_map[n] >= 0, and the RGCN
aggregation only survives for the (very few) nodes with history_map[n] < 0.

Strategy (memory-bound regime):
  - Shard destination nodes across 8 cores (6250 each); each core
    indirect-gathers its history rows into staging tiles, one tile per
    gather segment. Segment sizes are chosen so each segment's SWDGE
    descriptor generation (994ns fixed + 0.34ns/desc on the Pool engine)
    hides completely under the previous segment's DMA transfer — the DMA
    engines never idle between segments.
  - The history-index table is loaded in two pieces: a small head (just the
    first segment's indices, ~90ns transfer) so descriptor generation starts
    as early as possible, then the rest.
  - Three output DMAs (grouping the five gather segments) queue up behind
    the remaining gathers and drain back-to-back with no DMA idle; the
    grouping is chosen per dataset so the first output joins the DMA-engine
    queue after the last gather (the device services acquires in arrival
    order), and the DMA count puts the last output's completion semaphore
    on the highest DMAHW lane so the Tile exit drain retires its other lane
    waits early. SP/Activation sequencers are reserved for DMA; all compute
    runs on DVE/PE so a waiting output DMA never blocks the compute chain.
  - The globally-rare "no history" nodes are computed on every core
    (replicated tiny fp32 compute keeps the SPMD program identical): their
    incoming edges are bucketed into 16-node chunks; per 128-edge tile we
    indirect-gather source features (host-prepared halo) and accumulate
    Z^T[64, 128] += Xg^T @ S on the tensor engine, where S is a (relation,
    node-rank) one-hot built on the vector engine. Relation weights +
    self-loop + bias are applied with small matmuls. Computed rows are
    routed to their data-dependent staging positions with one-hot selector
    matmuls and overlaid with a plain add (history_buffer is uploaded with
    one extra zeros row and invalid nodes' gather indices point at it, so
    the gathered value under an invalid row is 0); the overlay finishes
    (~10us) long before the affected output segment's DMA-engine slot comes
    up, so it never extends the critical path.
"""
import sys

sys.path.insert(0, "/opt/trn_rl_repo")

import numpy as np

import concourse.bacc as bacc
import concourse.tile as tile
import concourse.mybir as mybir
from concourse.bass_utils import run_bass_kernel_spmd

N_NODES = 50000
N_EDGES = 800000
CH = 64
N_REL = 8
BUF = 20000
N_CORES = 8
DPC = N_NODES // N_CORES            # 6250 dst nodes per core
NCOL = 49                           # staging columns (49 x 128 = 6272 >= 6250)
NPAD = NCOL * 128
GSEG = (8, 10, 18, 9, 4)            # gather segments (columns); see module doc
GOFF = (0, 8, 18, 36, 45)           # column offset of each segment
HEAD = GSEG[0]                      # columns covered by the hidx head load
# staging tiles / output DMAs: (cols, covering gather segments). Three outputs
# keep the DMA engines saturated, and together with the input loads they make
# exactly 8 (M>0) / 5 (M=0) HWDGE-lane DMAs, so the LAST output's completion
# semaphore lands on the highest used DMAHW lane — the Tile exit drain then
# retires all its other lane waits early instead of queueing behind this one.
# The first output must join the DMA-engine queue AFTER the last gather
# (arrival order is what the device services); an overlay add in its columns
# (waits a gather sem + compute) or covering segment 2 (waits its gather sem)
# provides that delay — chosen per dataset in _build_program.
CHUNK = 16                          # invalid nodes per compute chunk

_cache = {}


def _wrap16(a):
    """Flat index array -> [128, len/16] int16 wrapped layout (idx k at
    [k%16, k//16], replicated across the 8 gpsimd lanes)."""
    m = a.reshape(-1, 16).T.astype(np.int16)
    return np.tile(m, (8, 1)).copy()


def _host_prep(x, W, loop_w, bias, history_buffer, src, dst, etypes, history_map):
    src = np.asarray(src)
    dst = np.asarray(dst)
    etypes = np.asarray(etypes)
    x = np.asarray(x, dtype=np.float32)
    hm = np.asarray(history_map)
    hbz = np.zeros((BUF + 1, CH), np.float32)
    hbz[:BUF] = np.asarray(history_buffer, np.float32)  # row BUF stays zero

    # --- globally-rare invalid (no-history) nodes: replicated tiny compute ---
    inv_nodes = np.where(hm < 0)[0]              # sorted
    M = len(inv_nodes)
    NCHUNK = -(-M // CHUNK) if M > 0 else 0
    MP = max(CHUNK, NCHUNK * CHUNK)              # scratch rows (>=16)

    chunk_tiles = ()
    Tinv = 0
    xg_list = []
    srk_list = []
    if M > 0:
        grank = np.full(N_NODES, -1, np.int64)
        grank[inv_nodes] = np.arange(M)
        emask = grank[dst] >= 0
        e_src = src[emask]
        e_et = etypes[emask]
        e_rank = grank[dst[emask]]
        e_chunk = e_rank // CHUNK
        e_col = e_et * CHUNK + (e_rank % CHUNK)  # one-hot col within chunk

        # host-side halo of the invalid edges' source features (the
        # sharding hint's "halo of remote source features"): per 128-edge
        # tile, a [128, CH] f32 block; pad edges are zero rows.
        nt = []
        for ch in range(NCHUNK):
            m = e_chunk == ch
            cnt = int(m.sum())
            n = -(-cnt // 128) if cnt else 0
            srkv = np.zeros(n * 128, np.float32)
            srkv[:cnt] = e_col[m]
            xgv = np.zeros((n * 128, CH), np.float32)
            xgv[:cnt] = x[e_src[m]]
            for t in range(n):
                srk_list.append(srkv[t * 128:(t + 1) * 128])
                xg_list.append(xgv[t * 128:(t + 1) * 128])
            nt.append(n)
        chunk_tiles = tuple(nt)
        Tinv = len(srk_list)

    # union (over cores) of staging columns that hold an invalid node —
    # only these columns need the computed-row overlay
    if M:
        cols_used = sorted(set(((inv_nodes % DPC) // 128).tolist()))
    else:
        cols_used = []

    meta = {
        "M": M, "NCHUNK": NCHUNK, "MP": MP, "Tinv": Tinv,
        "chunk_tiles": chunk_tiles, "cols_used": tuple(cols_used),
    }

    shared = {"hbuf": hbz}
    TinvP = max(1, Tinv)
    if M > 0:
        xg_halo = np.zeros((128, TinvP, CH), np.float32)
        for t, blk in enumerate(xg_list):
            xg_halo[:, t, :] = blk
        srk = np.zeros((128, TinvP), np.float32)
        if Tinv:
            srk[:, :Tinv] = np.stack(srk_list, axis=1)
        lwa = np.zeros((128, CH), np.float32)
        lwa[:CH] = np.asarray(loop_w, np.float32)
        lwa[CH] = np.asarray(bias, np.float32)
        iota = np.tile(np.arange(128, dtype=np.float32)[None, :], (128, 1)).copy()
        xti = np.zeros((128, MP), np.float32)
        xti[:CH, :M] = x[inv_nodes].T
        xti[CH, :M] = 1.0
        Wsb = np.zeros((64, N_REL * CH), np.float32)
        for r in range(N_REL):
            Wsb[:, r * CH:(r + 1) * CH] = np.asarray(W[r], np.float32)
        # merge the small f32 constants into one array (fewer DMAs):
        # [srk | iota(128) | lwa | xti | wsb(rows 0:64)]
        cmega = np.zeros((128, TinvP + 128 + CH + MP + N_REL * CH), np.float32)
        o = 0
        cmega[:, o:o + TinvP] = srk; o += TinvP
        cmega[:, o:o + 128] = iota; o += 128
        cmega[:, o:o + CH] = lwa; o += CH
        cmega[:, o:o + MP] = xti; o += MP
        cmega[:64, o:o + N_REL * CH] = Wsb
        shared["cmega"] = cmega
        shared["xg"] = xg_halo

    in_maps = []
    for c in range(N_CORES):
        hm_loc = np.zeros(NPAD, np.int64)
        hm_loc[:DPC] = hm[c * DPC:(c + 1) * DPC]
        hidx = np.where(hm_loc < 0, BUF, hm_loc)  # invalid -> the zeros row
        wrapped = _wrap16(hidx)
        im = {
            **shared,
            "hidx_head": wrapped[:, :HEAD * 8].copy(),
            "hidx_rest": wrapped[:, HEAD * 8:].copy(),
        }
        if M > 0:
            # selector shipped only for the staging columns in cols_used
            NCU = max(len(cols_used), 1)
            sel = np.zeros((CHUNK, max(NCHUNK, 1) * NCU * 128), np.float32)
            gr = grank[c * DPC:(c + 1) * DPC]
            loc_inv = np.where(gr >= 0)[0]
            col_pos = {cb: i for i, cb in enumerate(cols_used)}
            for n in loc_inv:
                rr = int(gr[n])
                i = col_pos[n // 128]
                sel[rr % CHUNK,
                    ((rr // CHUNK) * NCU + i) * 128 + (n % 128)] = 1.0
            im["sel"] = sel
        in_maps.append(im)
    return meta, in_maps


def _build_program(meta):
    M, NCHUNK, MP = meta["M"], meta["NCHUNK"], meta["MP"]
    Tinv = meta["Tinv"]
    TinvP = max(1, Tinv)
    CMW = TinvP + 128 + CH + MP + N_REL * CH
    NCU = max(len(meta["cols_used"]), 1)

    if M > 0 and any(cb < GOFF[2] for cb in meta["cols_used"]):
        # overlay in the first two segments delays output A past the last
        # gather's queue entry
        oseg = ((18, (0, 1)), (18, (2,)), (13, (3, 4)))
    else:
        # no early overlay: output A covers segment 2 so its readiness
        # (G2's completion) lands between the last gather's queue entry
        # and the end of the gather stream
        oseg = ((36, (0, 1, 2)), (9, (3,)), (4, (4,)))

    nc = bacc.Bacc("TRN2", target_bir_lowering=False, debug=False,
                   num_devices=N_CORES,
                   # the gathers emit ~6.3k SWDGE descriptors; the default
                   # 1024-descriptor ring would force a mid-kernel drain
                   dynamic_dma_scratch_size=1 << 17)
    dt = mybir.dt
    d_hbuf = nc.dram_tensor("hbuf", [BUF + 1, CH], dt.float32,
                            kind="ExternalInput")
    d_hh = nc.dram_tensor("hidx_head", [128, HEAD * 8], dt.int16,
                          kind="ExternalInput")
    d_hr = nc.dram_tensor("hidx_rest", [128, (NCOL - HEAD) * 8], dt.int16,
                          kind="ExternalInput")
    d_out = nc.dram_tensor("out", [128, NCOL, CH], dt.float32,
                           kind="ExternalOutput")
    if M > 0:
        d_cm = nc.dram_tensor("cmega", [128, CMW], dt.float32,
                              kind="ExternalInput")
        d_xg = nc.dram_tensor("xg", [128, TinvP, CH], dt.float32,
                              kind="ExternalInput")
        d_sel = nc.dram_tensor("sel", [CHUNK, max(NCHUNK, 1) * NCU * 128],
                               dt.float32, kind="ExternalInput")

    # map global staging column -> (segment, local column)
    def seg_of(cb):
        for s in range(len(GSEG) - 1, -1, -1):
            if cb >= GOFF[s]:
                return s, cb - GOFF[s]
        raise AssertionError(cb)

    with tile.TileContext(nc) as tc:
        with (
            tc.tile_pool(name="const", bufs=1) as cpool,
            tc.tile_pool(name="s", bufs=2) as spool,
            tc.tile_pool(name="pz", bufs=2, space="PSUM") as pzpool,
            tc.tile_pool(name="po", bufs=2, space="PSUM") as popool,
            tc.tile_pool(name="pov", bufs=4, space="PSUM") as povpool,
        ):
            hh_sb = cpool.tile([128, HEAD * 8], dt.int16)
            hr_sb = cpool.tile([128, (NCOL - HEAD) * 8], dt.int16)
            otiles = [cpool.tile([128, oc, CH], dt.float32, name=f"otile{i}")
                      for i, (oc, _) in enumerate(oseg)]
            # per-gather-segment views into the output tiles
            stages = []
            for i, (oc, segs) in enumerate(oseg):
                base = GOFF[segs[0]]
                for s in segs:
                    stages.append((otiles[i], GOFF[s] - base))
            stages = [stages[s] for s in range(len(GSEG))]

            # hidx head first: its (tiny) transfer + desc-gen of segment 0
            # gates the whole DMA pipeline
            nc.sync.dma_start(hh_sb[:], d_hh[:])
            nc.scalar.dma_start(hr_sb[:], d_hr[:])

            if M > 0:
                cm_sb = cpool.tile([128, CMW], dt.float32)
                xg_sb = cpool.tile([128, TinvP, CH], dt.float32)
                sel_sb = cpool.tile([CHUNK, max(NCHUNK, 1) * NCU * 128],
                                    dt.float32)
                nc.sync.dma_start(cm_sb[:], d_cm[:])
                nc.scalar.dma_start(xg_sb[:], d_xg[:])
                nc.sync.dma_start(sel_sb[:], d_sel[:])

            # history gathers, one per segment; desc-gen of segment s+1
            # hides under the transfer of segment s
            for s, g in enumerate(GSEG):
                ni = g * 128
                if s == 0:
                    idx = hh_sb[:, 0:g * 8]
                else:
                    o8 = (GOFF[s] - HEAD) * 8
                    idx = hr_sb[:, o8:o8 + g * 8]
                tile_sb, lc0 = stages[s]
                nc.gpsimd.dma_gather(
                    tile_sb[:, lc0:lc0 + g, :], d_hbuf[:], idx,
                    num_idxs=ni, num_idxs_reg=ni,
                    elem_size=CH, single_packet=False,
                )

            if M > 0:
                o = 0
                srk_sb = cm_sb[:, 0:TinvP]; o = TinvP
                iota_sb = cm_sb[:, o:o + 128]; o += 128
                lwa_sb = cm_sb[:, o:o + CH]; o += CH
                xti_sb = cm_sb[:, o:o + MP]; o += MP
                wsb_o = o

                gt = 0
                cps = []
                for ch in range(NCHUNK):
                    ntot = meta["chunk_tiles"][ch]
                    if ntot:
                        pz = pzpool.tile([64, 128], dt.float32, tag="pz",
                                         name=f"pz_{ch}")
                        for i in range(ntot):
                            S = spool.tile([128, 128], dt.float32, tag="S",
                                           name=f"S_{ch}_{i}")
                            nc.vector.tensor_scalar(
                                S[:], iota_sb, srk_sb[:, gt:gt + 1], None,
                                mybir.AluOpType.is_equal,
                            )
                            nc.tensor.matmul(pz[:], xg_sb[:, gt, :], S[:],
                                             start=(i == 0),
                                             stop=(i == ntot - 1))
                            gt += 1
                        zt = spool.tile([64, 128], dt.float32, tag="zt",
                                        name=f"zt_{ch}")
                        nc.vector.tensor_copy(zt[:], pz[:])
                    po = popool.tile([CHUNK, CH], dt.float32, tag="po",
                                     name=f"po_{ch}")
                    nc.tensor.matmul(po[:], xti_sb[:, ch * CHUNK:(ch + 1) * CHUNK],
                                     lwa_sb, start=True, stop=(ntot == 0))
                    if ntot:
                        for r in range(N_REL):
                            nc.tensor.matmul(
                                po[:], zt[:, r * CHUNK:(r + 1) * CHUNK],
                                cm_sb[0:64, wsb_o + r * CH:wsb_o + (r + 1) * CH],
                                start=False, stop=(r == N_REL - 1),
                            )
                    cp = cpool.tile([CHUNK, CH], dt.float32, name=f"cp_{ch}")
                    nc.vector.tensor_copy(cp[:], po[:])
                    cps.append(cp)

                # route computed rows to their positions; only columns that
                # hold an invalid node on some core need the overlay
                for i, cb in enumerate(meta["cols_used"]):
                    pov = povpool.tile([128, CH], dt.float32, tag="pov",
                                       name=f"pov_{cb}")
                    for ch in range(NCHUNK):
                        nc.tensor.matmul(
                            pov[:],
                            sel_sb[:, (ch * NCU + i) * 128:
                                   (ch * NCU + i) * 128 + 128],
                            cps[ch][:], start=(ch == 0),
                            stop=(ch == NCHUNK - 1),
                        )
                    s, lc = seg_of(cb)
                    tile_sb, lc0 = stages[s]
                    # valid rows gathered real history and pov is 0 there;
                    # invalid rows gathered the zeros row — plain add works
                    nc.vector.scalar_tensor_tensor(
                        tile_sb[:, lc0 + lc, :], tile_sb[:, lc0 + lc, :], 0.0,
                        pov[:], mybir.AluOpType.add, mybir.AluOpType.add)

            # output DMAs: issued in column order so they queue behind the
            # remaining gathers and drain back-to-back
            out_engs = (nc.scalar, nc.sync)
            c0 = 0
            for i, (oc, _) in enumerate(oseg):
                out_engs[i % 2].dma_start(d_out[:, c0:c0 + oc, :],
                                          otiles[i][:])
                c0 += oc
    nc.compile()
    return nc


def _prog_key(meta):
    return ("prog", meta["M"], meta["NCHUNK"], meta["Tinv"],
            meta["chunk_tiles"], meta["cols_used"])


def _run(inputs, trace=False):
    meta, in_maps = _host_prep(**inputs)
    key = _prog_key(meta)
    if key not in _cache:
        _cache[key] = _build_program(meta)
    nc = _cache[key]
    res = run_bass_kernel_spmd(nc, in_maps, list(range(N_CORES)), trace=trace)
    out = np.concatenate(
        [res.results[c]["out"].transpose(1, 0, 2).reshape(NPAD, CH)[:DPC]
         for c in range(N_CORES)], axis=0
    ).astype(np.float32)
    return out, res


def kernel(**inputs):
    out, _ = _run(inputs)
    return out


# revision 19
# speedup vs baseline: 1.0016x; 1.0016x over previous
"""RGCN-with-history (DGL RelGraphConv + history splice) on 8 TRN2 NeuronCores.

Key structural fact: the history splice dominates — out[n] is an exact copy of
history_buffer[history_map[n]] wherever history_map[n] >= 0, and the RGCN
aggregation only survives for the (very few) nodes with history_map[n] < 0.

Strategy (memory-bound regime):
  - Shard destination nodes across 8 cores (6250 each); each core
    indirect-gathers its history rows into staging tiles, one tile per
    gather segment. Segment sizes are chosen so each segment's SWDGE
    descriptor generation (994ns fixed + 0.34ns/desc on the Pool engine)
    hides completely under the previous segment's DMA transfer — the DMA
    engines never idle between segments.
  - The history-index table is loaded in two pieces: a small head (just the
    first segment's indices, ~90ns transfer) so descriptor generation starts
    as early as possible, then the rest.
  - Three output DMAs (grouping the five gather segments) queue up behind
    the remaining gathers and drain back-to-back with no DMA idle; the
    grouping is chosen per dataset so the first output joins the DMA-engine
    queue after the last gather (the device services acquires in arrival
    order), and the DMA count puts the last output's completion semaphore
    on the highest DMAHW lane so the Tile exit drain retires its other lane
    waits early. SP/Activation sequencers are reserved for DMA; all compute
    runs on DVE/PE so a waiting output DMA never blocks the compute chain.
  - The globally-rare "no history" nodes are computed on every core
    (replicated tiny fp32 compute keeps the SPMD program identical): their
    incoming edges are bucketed into 16-node chunks; per 128-edge tile we
    indirect-gather source features (host-prepared halo) and accumulate
    Z^T[64, 128] += Xg^T @ S on the tensor engine, where S is a (relation,
    node-rank) one-hot built on the vector engine. Relation weights +
    self-loop + bias are applied with small matmuls. Computed rows are
    routed to their data-dependent staging positions with one-hot selector
    matmuls and overlaid with a plain add (history_buffer is uploaded with
    one extra zeros row and invalid nodes' gather indices point at it, so
    the gathered value under an invalid row is 0); the overlay finishes
    (~10us) long before the affected output segment's DMA-engine slot comes
    up, so it never extends the critical path.
"""
import sys

sys.path.insert(0, "/opt/trn_rl_repo")

import numpy as np

import concourse.bacc as bacc
import concourse.tile as tile
import concourse.mybir as mybir
from concourse.bass_utils import run_bass_kernel_spmd

N_NODES = 50000
N_EDGES = 800000
CH = 64
N_REL = 8
BUF = 20000
N_CORES = 8
DPC = N_NODES // N_CORES            # 6250 dst nodes per core
NCOL = 49                           # staging columns (49 x 128 = 6272 >= 6250)
NPAD = NCOL * 128
GSEG = (8, 10, 18, 9, 4)            # gather segments (columns); see module doc
GOFF = (0, 8, 18, 36, 45)           # column offset of each segment
HEAD = GSEG[0]                      # columns covered by the hidx head load
# staging tiles / output DMAs: (cols, covering gather segments). Three outputs
# keep the DMA engines saturated, and together with the input loads they make
# exactly 8 (M>0) / 5 (M=0) HWDGE-lane DMAs, so the LAST output's completion
# semaphore lands on the highest used DMAHW lane — the Tile exit drain then
# retires all its other lane waits early instead of queueing behind this one.
# The first output must join the DMA-engine queue AFTER the last gather
# (arrival order is what the device services); an overlay add in its columns
# (waits a gather sem + compute) or covering segment 2 (waits its gather sem)
# provides that delay — chosen per dataset in _build_program.
CHUNK = 16                          # invalid nodes per compute chunk

_cache = {}


def _wrap16(a):
    """Flat index array -> [128, len/16] int16 wrapped layout (idx k at
    [k%16, k//16], replicated across the 8 gpsimd lanes)."""
    m = a.reshape(-1, 16).T.astype(np.int16)
    return np.tile(m, (8, 1)).copy()


def _host_prep(x, W, loop_w, bias, history_buffer, src, dst, etypes, history_map):
    src = np.asarray(src)
    dst = np.asarray(dst)
    etypes = np.asarray(etypes)
    x = np.asarray(x, dtype=np.float32)
    hm = np.asarray(history_map)
    hbz = np.zeros((BUF + 1, CH), np.float32)
    hbz[:BUF] = np.asarray(history_buffer, np.float32)  # row BUF stays zero

    # --- globally-rare invalid (no-history) nodes: replicated tiny compute ---
    inv_nodes = np.where(hm < 0)[0]              # sorted
    M = len(inv_nodes)
    NCHUNK = -(-M // CHUNK) if M > 0 else 0
    MP = max(CHUNK, NCHUNK * CHUNK)              # scratch rows (>=16)

    chunk_tiles = ()
    Tinv = 0
    xg_list = []
    srk_list = []
    if M > 0:
        grank = np.full(N_NODES, -1, np.int64)
        grank[inv_nodes] = np.arange(M)
        emask = grank[dst] >= 0
        e_src = src[emask]
        e_et = etypes[emask]
        e_rank = grank[dst[emask]]
        e_chunk = e_rank // CHUNK
        e_col = e_et * CHUNK + (e_rank % CHUNK)  # one-hot col within chunk

        # host-side halo of the invalid edges' source features (the
        # sharding hint's "halo of remote source features"): per 128-edge
        # tile, a [128, CH] f32 block; pad edges are zero rows.
        nt = []
        for ch in range(NCHUNK):
            m = e_chunk == ch
            cnt = int(m.sum())
            n = -(-cnt // 128) if cnt else 0
            srkv = np.zeros(n * 128, np.float32)
            srkv[:cnt] = e_col[m]
            xgv = np.zeros((n * 128, CH), np.float32)
            xgv[:cnt] = x[e_src[m]]
            for t in range(n):
                srk_list.append(srkv[t * 128:(t + 1) * 128])
                xg_list.append(xgv[t * 128:(t + 1) * 128])
            nt.append(n)
        chunk_tiles = tuple(nt)
        Tinv = len(srk_list)

    # union (over cores) of staging columns that hold an invalid node —
    # only these columns need the computed-row overlay
    if M:
        cols_used = sorted(set(((inv_nodes % DPC) // 128).tolist()))
    else:
        cols_used = []

    meta = {
        "M": M, "NCHUNK": NCHUNK, "MP": MP, "Tinv": Tinv,
        "chunk_tiles": chunk_tiles, "cols_used": tuple(cols_used),
    }

    shared = {"hbuf": hbz}
    TinvP = max(1, Tinv)
    if M > 0:
        xg_halo = np.zeros((128, TinvP, CH), np.float32)
        for t, blk in enumerate(xg_list):
            xg_halo[:, t, :] = blk
        srk = np.zeros((128, TinvP), np.float32)
        if Tinv:
            srk[:, :Tinv] = np.stack(srk_list, axis=1)
        lwa = np.zeros((128, CH), np.float32)
        lwa[:CH] = np.asarray(loop_w, np.float32)
        lwa[CH] = np.asarray(bias, np.float32)
        iota = np.tile(np.arange(128, dtype=np.float32)[None, :], (128, 1)).copy()
        xti = np.zeros((128, MP), np.float32)
        xti[:CH, :M] = x[inv_nodes].T
        xti[CH, :M] = 1.0
        Wsb = np.zeros((64, N_REL * CH), np.float32)
        for r in range(N_REL):
            Wsb[:, r * CH:(r + 1) * CH] = np.asarray(W[r], np.float32)
        # merge the small f32 constants into one array (fewer DMAs):
        # [srk | iota(128) | lwa | xti | wsb(rows 0:64)]
        cmega = np.zeros((128, TinvP + 128 + CH + MP + N_REL * CH), np.float32)
        o = 0
        cmega[:, o:o + TinvP] = srk; o += TinvP
        cmega[:, o:o + 128] = iota; o += 128
        cmega[:, o:o + CH] = lwa; o += CH
        cmega[:, o:o + MP] = xti; o += MP
        cmega[:64, o:o + N_REL * CH] = Wsb
        shared["cmega"] = cmega
        shared["xg"] = xg_halo

    in_maps = []
    for c in range(N_CORES):
        hm_loc = np.zeros(NPAD, np.int64)
        hm_loc[:DPC] = hm[c * DPC:(c + 1) * DPC]
        hidx = np.where(hm_loc < 0, BUF, hm_loc)  # invalid -> the zeros row
        wrapped = _wrap16(hidx)
        im = {
            **shared,
            "hidx_head": wrapped[:, :HEAD * 8].copy(),
            "hidx_rest": wrapped[:, HEAD * 8:].copy(),
        }
        if M > 0:
            # selector shipped only for the staging columns in cols_used
            NCU = max(len(cols_used), 1)
            sel = np.zeros((CHUNK, max(NCHUNK, 1) * NCU * 128), np.float32)
            gr = grank[c * DPC:(c + 1) * DPC]
            loc_inv = np.where(gr >= 0)[0]
            col_pos = {cb: i for i, cb in enumerate(cols_used)}
            for n in loc_inv:
                rr = int(gr[n])
                i = col_pos[n // 128]
                sel[rr % CHUNK,
                    ((rr // CHUNK) * NCU + i) * 128 + (n % 128)] = 1.0
            im["sel"] = sel
        in_maps.append(im)
    return meta, in_maps


def _build_program(meta):
    M, NCHUNK, MP = meta["M"], meta["NCHUNK"], meta["MP"]
    Tinv = meta["Tinv"]
    TinvP = max(1, Tinv)
    CMW = TinvP + 128 + CH + MP + N_REL * CH
    NCU = max(len(meta["cols_used"]), 1)

    if M > 0 and any(cb < GOFF[2] for cb in meta["cols_used"]):
        # overlay in the first two segments delays output A past the last
        # gather's queue entry
        oseg = ((18, (0, 1)), (18, (2,)), (13, (3, 4)))
    else:
        # no early overlay: output A covers segment 2 so its readiness
        # (G2's completion) lands between the last gather's queue entry
        # and the end of the gather stream
        oseg = ((36, (0, 1, 2)), (9, (3,)), (4, (4,)))

    nc = bacc.Bacc("TRN2", target_bir_lowering=False, debug=False,
                   num_devices=N_CORES,
                   # the gathers emit ~6.3k SWDGE descriptors; the default
                   # 1024-descriptor ring would force a mid-kernel drain
                   dynamic_dma_scratch_size=1 << 17)
    dt = mybir.dt
    d_hbuf = nc.dram_tensor("hbuf", [BUF + 1, CH], dt.float32,
                            kind="ExternalInput")
    d_hh = nc.dram_tensor("hidx_head", [128, HEAD * 8], dt.int16,
                          kind="ExternalInput")
    d_hr = nc.dram_tensor("hidx_rest", [128, (NCOL - HEAD) * 8], dt.int16,
                          kind="ExternalInput")
    d_out = nc.dram_tensor("out", [128, NCOL, CH], dt.float32,
                           kind="ExternalOutput")
    if M > 0:
        d_cm = nc.dram_tensor("cmega", [128, CMW], dt.float32,
                              kind="ExternalInput")
        d_xg = nc.dram_tensor("xg", [128, TinvP, CH], dt.float32,
                              kind="ExternalInput")
        d_sel = nc.dram_tensor("sel", [CHUNK, max(NCHUNK, 1) * NCU * 128],
                               dt.float32, kind="ExternalInput")

    # map global staging column -> (segment, local column)
    def seg_of(cb):
        for s in range(len(GSEG) - 1, -1, -1):
            if cb >= GOFF[s]:
                return s, cb - GOFF[s]
        raise AssertionError(cb)

    with tile.TileContext(nc) as tc:
        with (
            tc.tile_pool(name="const", bufs=1) as cpool,
            tc.tile_pool(name="s", bufs=2) as spool,
            tc.tile_pool(name="pz", bufs=2, space="PSUM") as pzpool,
            tc.tile_pool(name="po", bufs=2, space="PSUM") as popool,
            tc.tile_pool(name="pov", bufs=4, space="PSUM") as povpool,
        ):
            hh_sb = cpool.tile([128, HEAD * 8], dt.int16)
            hr_sb = cpool.tile([128, (NCOL - HEAD) * 8], dt.int16)
            otiles = [cpool.tile([128, oc, CH], dt.float32, name=f"otile{i}")
                      for i, (oc, _) in enumerate(oseg)]
            # per-gather-segment views into the output tiles
            stages = []
            for i, (oc, segs) in enumerate(oseg):
                base = GOFF[segs[0]]
                for s in segs:
                    stages.append((otiles[i], GOFF[s] - base))
            stages = [stages[s] for s in range(len(GSEG))]

            # hidx head first: its (tiny) transfer + desc-gen of segment 0
            # gates the whole DMA pipeline
            nc.sync.dma_start(hh_sb[:], d_hh[:])
            nc.scalar.dma_start(hr_sb[:], d_hr[:])

            if M > 0:
                cm_sb = cpool.tile([128, CMW], dt.float32)
                xg_sb = cpool.tile([128, TinvP, CH], dt.float32)
                sel_sb = cpool.tile([CHUNK, max(NCHUNK, 1) * NCU * 128],
                                    dt.float32)
                nc.sync.dma_start(cm_sb[:], d_cm[:])
                nc.scalar.dma_start(xg_sb[:], d_xg[:])
                nc.sync.dma_start(sel_sb[:], d_sel[:])

            # pad staging slots (nodes 6250..6271) are never gathered —
            # zero that whole column up front (the gather then overwrites its
            # real rows) so the host-discarded pad output bytes are defined
            pt, plc = stages[len(GSEG) - 1]
            nc.vector.memset(pt[:, plc + GSEG[-1] - 1, :], 0.0)

            # history gathers, one per segment; desc-gen of segment s+1
            # hides under the transfer of segment s. The last segment stops
            # at the 6250 real rows instead of the padded 6272.
            for s, g in enumerate(GSEG):
                ni = min(g * 128, DPC - GOFF[s] * 128)
                if s == 0:
                    idx = hh_sb[:, 0:g * 8]
                else:
                    o8 = (GOFF[s] - HEAD) * 8
                    idx = hr_sb[:, o8:o8 + g * 8]
                tile_sb, lc0 = stages[s]
                nc.gpsimd.dma_gather(
                    tile_sb[:, lc0:lc0 + g, :], d_hbuf[:], idx,
                    num_idxs=ni, num_idxs_reg=ni,
                    elem_size=CH, single_packet=False,
                )

            if M > 0:
                o = 0
                srk_sb = cm_sb[:, 0:TinvP]; o = TinvP
                iota_sb = cm_sb[:, o:o + 128]; o += 128
                lwa_sb = cm_sb[:, o:o + CH]; o += CH
                xti_sb = cm_sb[:, o:o + MP]; o += MP
                wsb_o = o

                gt = 0
                cps = []
                for ch in range(NCHUNK):
                    ntot = meta["chunk_tiles"][ch]
                    if ntot:
                        pz = pzpool.tile([64, 128], dt.float32, tag="pz",
                                         name=f"pz_{ch}")
                        for i in range(ntot):
                            S = spool.tile([128, 128], dt.float32, tag="S",
                                           name=f"S_{ch}_{i}")
                            nc.vector.tensor_scalar(
                                S[:], iota_sb, srk_sb[:, gt:gt + 1], None,
                                mybir.AluOpType.is_equal,
                            )
                            nc.tensor.matmul(pz[:], xg_sb[:, gt, :], S[:],
                                             start=(i == 0),
                                             stop=(i == ntot - 1))
                            gt += 1
                        zt = spool.tile([64, 128], dt.float32, tag="zt",
                                        name=f"zt_{ch}")
                        nc.vector.tensor_copy(zt[:], pz[:])
                    po = popool.tile([CHUNK, CH], dt.float32, tag="po",
                                     name=f"po_{ch}")
                    nc.tensor.matmul(po[:], xti_sb[:, ch * CHUNK:(ch + 1) * CHUNK],
                                     lwa_sb, start=True, stop=(ntot == 0))
                    if ntot:
                        for r in range(N_REL):
                            nc.tensor.matmul(
                                po[:], zt[:, r * CHUNK:(r + 1) * CHUNK],
                                cm_sb[0:64, wsb_o + r * CH:wsb_o + (r + 1) * CH],
                                start=False, stop=(r == N_REL - 1),
                            )
                    cp = cpool.tile([CHUNK, CH], dt.float32, name=f"cp_{ch}")
                    nc.vector.tensor_copy(cp[:], po[:])
                    cps.append(cp)

                # route computed rows to their positions; only columns that
                # hold an invalid node on some core need the overlay
                for i, cb in enumerate(meta["cols_used"]):
                    pov = povpool.tile([128, CH], dt.float32, tag="pov",
                                       name=f"pov_{cb}")
                    for ch in range(NCHUNK):
                        nc.tensor.matmul(
                            pov[:],
                            sel_sb[:, (ch * NCU + i) * 128:
                                   (ch * NCU + i) * 128 + 128],
                            cps[ch][:], start=(ch == 0),
                            stop=(ch == NCHUNK - 1),
                        )
                    s, lc = seg_of(cb)
                    tile_sb, lc0 = stages[s]
                    # valid rows gathered real history and pov is 0 there;
                    # invalid rows gathered the zeros row — plain add works
                    nc.vector.scalar_tensor_tensor(
                        tile_sb[:, lc0 + lc, :], tile_sb[:, lc0 + lc, :], 0.0,
                        pov[:], mybir.AluOpType.add, mybir.AluOpType.add)

            # output DMAs: issued in column order so they queue behind the
            # remaining gathers and drain back-to-back
            out_engs = (nc.scalar, nc.sync)
            c0 = 0
            for i, (oc, _) in enumerate(oseg):
                out_engs[i % 2].dma_start(d_out[:, c0:c0 + oc, :],
                                          otiles[i][:])
                c0 += oc
    nc.compile()
    return nc


def _prog_key(meta):
    return ("prog", meta["M"], meta["NCHUNK"], meta["Tinv"],
            meta["chunk_tiles"], meta["cols_used"])


def _run(inputs, trace=False):
    meta, in_maps = _host_prep(**inputs)
    key = _prog_key(meta)
    if key not in _cache:
        _cache[key] = _build_program(meta)
    nc = _cache[key]
    res = run_bass_kernel_spmd(nc, in_maps, list(range(N_CORES)), trace=trace)
    out = np.concatenate(
        [res.results[c]["out"].transpose(1, 0, 2).reshape(NPAD, CH)[:DPC]
         for c in range(N_CORES)], axis=0
    ).astype(np.float32)
    return out, res


def kernel(**inputs):
    out, _ = _run(inputs)
    return out
